# revision 3
# baseline (speedup 1.0000x reference)
"""2-layer GAT on 8 Trainium2 NeuronCores (Bass/Tile) — v2 lane-major design.

Key ideas vs v1:
  * Lane-major edge layout: dst node <-> SBUF partition lane. Edges of dst j
    sit in column j of the gathered tile, so the segment softmax/aggregation
    is a per-partition reduction (PE identity-matmul accumulate over chunk
    slices into PSUM) — no selection-matrix build, no dl table.
  * Softmax scale invariance removes the per-edge dst gather entirely:
    w = exp(leaky(as+ad)-M) can be divided by exp(ad) per dst, giving
    w' = max(exp(as-M), exp(0.2*as-M)*G) with per-dst G = exp(-0.8*ad) —
    a per-partition broadcast in lane-major layout.
  * int16 gather indices address 32768 rows; full node table needs 50176.
    Tables are stored as 2-slot "pair rows" and gathered with
    elem_step = 2 rows, giving an even window and an odd window (idx = slot//2
    in both). Slot parity is made flexible by appending copies of the
    hottest-src nodes at opposite parity, so each dst lane can balance its
    edges across the two windows (pad entries point at dedicated pad rows
    whose attention logit is -100 => weight exactly 0).
  * 2 blocks share each gather call (fewer SWDGE fixed costs); big DMA
    descriptor ring so calls of ~2-3k indices fit.
"""

import sys

sys.path.insert(0, "/opt/trn_rl_repo")

import numpy as np

import concourse.bacc as bacc
import concourse.bass as bass
import concourse.mybir as mybir
import concourse.tile as tile

# ---------------- problem constants (hardcoded per task contract) -------------
N = 50000
F_IN = 128
HID = 16
HEADS = 8
CLASSES = 16
NEG = 0.2

N_CORES = 8
P = 128
BLOCKS = 49
NODES_PER_CORE = BLOCKS * P                # 6272
N_PAD = N_CORES * NODES_PER_CORE           # 50176
TROWS = 65536                              # table slots: primary+pads+copies
PAD_EVEN = N_PAD                           # 50176
PAD_ODD = N_PAD + 1                        # 50177
COPY0 = N_PAD + 2                          # 50178
M_SHIFT = 4.0
GMAX = 1024                                # max indices per dma_gather call (HW ring cap)
NQ = 4
SCRATCH = 16384                            # SWDGE ring (default; ucode ring is 1024 descs)

BROW = 256                                 # B table row elems (f16): h|as|ad|pad
CROW = 128                                 # C table row elems (f16): z|as2|ad2|pad

F16 = mybir.dt.float16
F32 = mybir.dt.float32
I16 = mybir.dt.int16

_cache = {}
_last_cfg = None
_last_inputs = None


# ---------------------------- host preprocessing -----------------------------

def _wrap16(a):
    """Index array [n] -> dma_gather SBUF layout [128, n/16]."""
    n = a.shape[0]
    assert n % 16 == 0
    w = a.reshape(n // 16, 16).T.astype(np.int16)
    return np.tile(w, (8, 1))


def _cumcount(keys):
    """For a sorted key array, position of each element within its key group."""
    n = len(keys)
    if n == 0:
        return np.zeros(0, np.int64)
    starts = np.r_[0, np.flatnonzero(np.diff(keys)) + 1]
    lens = np.diff(np.r_[starts, n])
    return np.arange(n) - np.repeat(starts, lens)


def host_prep(edge_index):
    # self-loops are handled densely on-device; gathers cover edge_index only
    src = edge_index[0].astype(np.int64)
    dst = edge_index[1].astype(np.int64)
    deg = np.bincount(dst, minlength=N_PAD)          # gather degree (no loop)
    outdeg = np.bincount(src, minlength=N_PAD)

    # 1. copy set: hottest src nodes get a second table row at opposite parity
    NCOPY = TROWS - N_PAD - 2                        # 15358 (7679 even+7679 odd)
    HALF = NCOPY // 2
    ctop = np.argsort(-outdeg, kind="stable")[:NCOPY]
    is_copy = np.zeros(N_PAD, bool)
    is_copy[ctop] = True
    strict = ~is_copy[src]
    s_src = src[strict]
    s_dst = dst[strict]

    # 2. node parity via damped discrepancy passes over strict edges:
    #    balance each dst's in-edge parity split
    par = (np.arange(N_PAD) & 1).astype(np.int64)
    rng_mask = ((np.arange(N_PAD) * 2654435761) % 100) < 50
    for it in range(20):
        sgn = 1 - 2 * par                            # +1 even, -1 odd
        imb = np.bincount(s_dst, weights=sgn[s_src], minlength=N_PAD)
        grad = np.bincount(s_src, weights=imb[s_dst], minlength=N_PAD)
        flip = (sgn * grad) > 1.5
        flip &= ~is_copy
        if it < 14:
            flip &= rng_mask if (it % 2 == 0) else ~rng_mask
        par[flip] ^= 1

    # repair exact parity quotas: non-copied must be 17409/17409,
    # copied 7679/7679 (even slots total 25088)
    NC_EVEN = N_PAD // 2 - HALF                      # 17409
    nonc = np.flatnonzero(~is_copy)
    nce = int((par[nonc] == 0).sum())
    if nce != NC_EVEN:
        want = 0 if nce < NC_EVEN else 1             # parity to flip INTO
        cand = nonc[par[nonc] != want]
        sgn = 1 - 2 * par
        imb = np.bincount(s_dst, weights=sgn[s_src], minlength=N_PAD)
        grad = np.bincount(s_src, weights=imb[s_dst], minlength=N_PAD)
        cost = -(sgn * grad)                         # prefer improving flips
        take = cand[np.argsort(cost[cand], kind="stable")[:abs(nce - NC_EVEN)]]
        par[take] ^= 1
    cop = np.flatnonzero(is_copy)
    ce_count = int((par[cop] == 0).sum())
    if ce_count != HALF:
        want = 0 if ce_count < HALF else 1
        cand = cop[par[cop] != want]
        par[cand[:abs(ce_count - HALF)]] ^= 1

    # 3. copy rows: even-slot copied node -> odd copy row, and vice versa
    copy_row = np.full(N_PAD, -1, np.int64)
    even_cop = cop[par[cop] == 0]
    odd_cop = cop[par[cop] == 1]
    copy_row[even_cop] = np.arange(COPY0 + 1, TROWS, 2)[:len(even_cop)]
    copy_row[odd_cop] = np.arange(COPY0, TROWS, 2)[:len(odd_cop)]

    # 4. strict per-dst parity counts (with final parities)
    sp_par = par[src]
    flex = is_copy[src]
    nE = np.bincount(dst[(~flex) & (sp_par == 0)], minlength=N_PAD)
    nO = np.bincount(dst[(~flex) & (sp_par == 1)], minlength=N_PAD)

    # 5. cores: deal each parity class round-robin by degree rank
    core_of = np.empty(N_PAD, np.int64)
    within = np.empty(N_PAD, np.int64)               # per-core per-parity index
    for p in (0, 1):
        nodes_p = np.flatnonzero(par == p)
        nodes_p = nodes_p[np.argsort(-deg[nodes_p], kind="stable")]
        core_of[nodes_p] = np.arange(len(nodes_p)) % N_CORES
        within[nodes_p] = np.arange(len(nodes_p)) // N_CORES
    slot_of_node = core_of * NODES_PER_CORE + within * 2 + par
    node_of_slot = np.empty(N_PAD, np.int64)
    node_of_slot[slot_of_node] = np.arange(N_PAD)

    # 6. bins: per core sort nodes by (deg desc, nE desc), chunk into 49x128;
    #    every core sorts by the same key -> aligned block profiles
    lane_node = np.empty((N_CORES, BLOCKS, P), np.int64)
    for k in range(N_CORES):
        nodes_k = np.flatnonzero(core_of == k)
        key = deg[nodes_k] * 100000 + nE[nodes_k]
        nodes_k = nodes_k[np.argsort(-key, kind="stable")]
        lane_node[k] = nodes_k.reshape(BLOCKS, P)
    blk_of_node = np.empty(N_PAD, np.int64)
    lane_of_node = np.empty(N_PAD, np.int64)
    blk_of_node[lane_node.reshape(-1)] = np.tile(
        np.repeat(np.arange(BLOCKS), P), N_CORES)
    lane_of_node[lane_node.reshape(-1)] = np.tile(np.arange(P), N_CORES * BLOCKS)

    # 7. per-block caps: C* = max(maxD, maxNE+maxNO); cE >= maxNE, cO >= maxNO
    maxD = np.zeros(BLOCKS, np.int64)
    maxNE = np.zeros(BLOCKS, np.int64)
    maxNO = np.zeros(BLOCKS, np.int64)
    np.maximum.at(maxD, blk_of_node, deg)
    np.maximum.at(maxNE, blk_of_node, nE)
    np.maximum.at(maxNO, blk_of_node, nO)
    Cstar = np.maximum(maxD, maxNE + maxNO)
    cE = np.maximum(maxNE, Cstar - maxNO)
    cO = Cstar - cE

    # 8. per-dst even-section count aE in [max(nE, d-cO), min(cE, d-nO)]
    d_n = deg
    lo = np.maximum(nE, d_n - cO[blk_of_node])
    hi = np.minimum(cE[blk_of_node], d_n - nO)
    aE_n = np.clip((d_n + 1) // 2, lo, hi)

    # 9. per-edge section: strict follow parity; flex fill even first
    sec = sp_par.copy()
    fidx = np.flatnonzero(flex)
    fperm = fidx[np.argsort(dst[fidx], kind="stable")]
    cum = _cumcount(dst[fperm])
    sec[fperm] = np.where(cum < (aE_n - nE)[dst[fperm]], 0, 1)

    # chunk index of each edge within its (dst, section) group
    key = dst * 2 + sec
    eperm = np.argsort(key, kind="stable")
    cc = _cumcount(key[eperm])
    chunk = np.empty(len(dst), np.int64)
    chunk[eperm] = cc

    kk = core_of[dst]
    bb = blk_of_node[dst]
    jj = lane_of_node[dst]
    ss = slot_of_node[src]
    row = np.where(par[src] == sec, ss, copy_row[src])
    idx16 = row >> 1
    PADE16 = PAD_EVEN >> 1
    PADO16 = PAD_ODD >> 1

    copy_rows = copy_row[cop]
    copy_slots = slot_of_node[cop]
    cE = cE.astype(np.int64)
    cO = cO.astype(np.int64)

    # pair layout: pairs of blocks (2t, 2t+1); block 48 alone
    pairs = [(2 * t, 2 * t + 1) for t in range(BLOCKS // 2)] + [(BLOCKS - 1,)]
    # per-pair section geometry (in chunks)
    pair_info = []
    si_cols = 0
    for pb in pairs:
        ev = [int(cE[b]) for b in pb]
        od = [int(cO[b]) for b in pb]
        n_ev = sum(ev) * P
        n_od = sum(od) * P
        pair_info.append(dict(blocks=pb, ev=ev, od=od, col0=si_cols,
                              n_ev=n_ev, n_od=n_od))
        si_cols += (n_ev + n_od) // 16

    # assemble si tables [cores][128, si_cols]
    si_all = np.empty((N_CORES, 128, si_cols), np.int16)
    pair_of_block = np.empty(BLOCKS, np.int64)
    posoff_ev = np.empty(BLOCKS, np.int64)   # chunk offset of block's ev sec in pair-call
    posoff_od = np.empty(BLOCKS, np.int64)
    for pi, info in enumerate(pair_info):
        off = 0
        for i, b in enumerate(info["blocks"]):
            pair_of_block[b] = pi
            posoff_ev[b] = off
            off += info["ev"][i]
        off2 = 0
        for i, b in enumerate(info["blocks"]):
            posoff_od[b] = off2
            off2 += info["od"][i]

    # flat per-core index arrays, default pad
    flat_ev = [np.full(info["n_ev"], PADE16, np.int16) for info in pair_info]
    flat_od = [np.full(info["n_od"], PADO16, np.int16) for info in pair_info]
    flat_ev = [np.tile(f[None], (N_CORES, 1)) for f in flat_ev]
    flat_od = [np.tile(f[None], (N_CORES, 1)) for f in flat_od]

    pii = pair_of_block[bb]
    pos = np.where(sec == 0,
                   (posoff_ev[bb] + chunk) * P + jj,
                   (posoff_od[bb] + chunk) * P + jj)
    for pi in range(len(pair_info)):
        m0 = (pii == pi) & (sec == 0)
        m1 = (pii == pi) & (sec == 1)
        flat_ev[pi][kk[m0], pos[m0]] = idx16[m0].astype(np.int16)
        flat_od[pi][kk[m1], pos[m1]] = idx16[m1].astype(np.int16)

    for k in range(N_CORES):
        parts = []
        for pi, info in enumerate(pair_info):
            parts.append(_wrap16(flat_ev[pi][k]))
            parts.append(_wrap16(flat_od[pi][k]))
        si_all[k] = np.concatenate(parts, axis=1)

    lane_slot = slot_of_node[lane_node]              # [8, 49, 128]
    return dict(slot_of_node=slot_of_node, node_of_slot=node_of_slot,
                copy_rows=copy_rows, copy_slots=copy_slots,
                lane_node=lane_node, lane_slot=lane_slot,
                cE=cE.tolist(), cO=cO.tolist(),
                pair_info=pair_info, si_cols=si_cols, si_all=si_all)


# ------------------------------- NEFF builders -------------------------------

def build_neff_a(reps=1):
    nc = bacc.Bacc()
    xT = nc.dram_tensor("xT", [P, NODES_PER_CORE], F16, kind="ExternalInput")
    w1e = nc.dram_tensor("w1e", [P, 144], F16, kind="ExternalInput")
    g_out = nc.dram_tensor("g_out", [NODES_PER_CORE, 144], F16, kind="ExternalOutput")
    with tile.TileContext(nc) as tc:
        with tc.tile_pool(name="sbuf", bufs=4) as pool, \
             tc.tile_pool(name="psum", bufs=4, space="PSUM") as pp:
            w1t = pool.tile([P, 144], F16)
            nc.sync.dma_start(w1t[:], w1e[:])

            def body():
                xt = pool.tile([P, NODES_PER_CORE], F16, tag="xt", name="xt")
                nc.sync.dma_start(xt[:], xT[:])
                for t in range(BLOCKS):
                    ps = pp.tile([P, 144], F32, tag="ps", space="PSUM", name="ps")
                    nc.tensor.matmul(out=ps[:], lhsT=xt[:, t * P:(t + 1) * P],
                                     rhs=w1t[:], start=True, stop=True)
                    gt = pool.tile([P, 144], F16, tag="gt", name="gt")
                    nc.vector.tensor_copy(out=gt[:], in_=ps[:])
                    nc.sync.dma_start(g_out[t * P:(t + 1) * P, :], gt[:])

            if reps == 1:
                body()
            else:
                with tc.For_i(0, reps, 1):
                    body()
    nc.finalize()
    return nc


def _gather_sections(nc, qrr, X, g_d, si, info):
    """Issue even+odd gathers for one pair into X tile."""
    n_ev, n_od = info["n_ev"], info["n_od"]
    ev_ch = n_ev // P
    row = X.shape[2]
    for base0 in range(0, n_ev, GMAX):
        n = min(GMAX, n_ev - base0)
        nc.gpsimd.dma_gather(
            out_ap=X[:, base0 // P:(base0 + n) // P, :],
            in_ap=g_d[:, 0, :],
            idxs_ap=si[:, base0 // 16:(base0 + n) // 16],
            num_idxs=n, num_idxs_reg=n, elem_size=row, elem_step=2 * row,
            queue_num=qrr())
    for base0 in range(0, n_od, GMAX):
        n = min(GMAX, n_od - base0)
        nc.gpsimd.dma_gather(
            out_ap=X[:, ev_ch + base0 // P:ev_ch + (base0 + n) // P, :],
            in_ap=g_d[:, 1, :],
            idxs_ap=si[:, (n_ev + base0) // 16:(n_ev + base0 + n) // 16],
            num_idxs=n, num_idxs_reg=n, elem_size=row, elem_step=2 * row,
            queue_num=qrr())


def build_neff_b(cfg, reps=1):
    nc = bacc.Bacc(num_swdge_queues=NQ, dynamic_dma_scratch_size=SCRATCH)
    pair_info = cfg["pair_info"]
    si_cols = cfg["si_cols"]
    g_d = nc.dram_tensor("g", [TROWS // 2, 2, BROW], F16, kind="ExternalInput")
    si_d = nc.dram_tensor("si", [128, si_cols], I16, kind="ExternalInput")
    own_d = nc.dram_tensor("own", [BLOCKS, P, 144], F16, kind="ExternalInput")
    identf_d = nc.dram_tensor("identf", [P, P], F16, kind="ExternalInput")
    ident32_d = nc.dram_tensor("ident32", [P, P], F32, kind="ExternalInput")
    b1r_d = nc.dram_tensor("b1r", [P, P], F32, kind="ExternalInput")
    w2e_d = nc.dram_tensor("w2e", [P, 18], F16, kind="ExternalInput")
    g2_out = nc.dram_tensor("g2_out", [BLOCKS, P, 18], F16, kind="ExternalOutput")

    qctr = [0]

    def qrr():
        qctr[0] = (qctr[0] + 1) % NQ
        return qctr[0]

    with tile.TileContext(nc) as tc:
        with tc.tile_pool(name="sbuf", bufs=2) as pool, \
             tc.tile_pool(name="psum", bufs=2, space="PSUM") as pp:
            identf = pool.tile([P, P], F16)
            nc.sync.dma_start(identf[:], identf_d[:])
            ident32 = pool.tile([P, P], F32)
            nc.sync.dma_start(ident32[:], ident32_d[:])
            b1r = pool.tile([P, P], F32)
            nc.sync.dma_start(b1r[:], b1r_d[:])
            w2e = pool.tile([P, 18], F16)
            nc.sync.dma_start(w2e[:], w2e_d[:])
            mshift = pool.tile([P, 1], F32)
            nc.gpsimd.memset(mshift[:], -M_SHIFT)

            def body():
                for info in pair_info:
                    tot_ch = (info["n_ev"] + info["n_od"]) // P
                    ev_ch = info["n_ev"] // P
                    X = pool.tile([P, tot_ch, BROW], F16, tag="X")
                    si = pool.tile([P, (info["n_ev"] + info["n_od"]) // 16], I16, tag="si")
                    nc.sync.dma_start(si[:], si_d[:, info["col0"]:info["col0"] + si.shape[1]])
                    _gather_sections(nc, qrr, X, g_d, si, info)

                    evo = 0
                    odo = ev_ch
                    for i, b in enumerate(info["blocks"]):
                        ce, co = info["ev"][i], info["od"][i]
                        ch = ce + co
                        ranges = [(evo, 0, ce), (odo, ce, co)]  # (Xoff, rhsoff, len)
                        own = pool.tile([P, 144], F16, tag="own")
                        nc.sync.dma_start(own[:], own_d[b])
                        G8 = pool.tile([P, HEADS], F16, tag="G8")
                        nc.scalar.activation(G8[:], own[:, 136:144],
                                             mybir.ActivationFunctionType.Exp,
                                             scale=-0.8)
                        rhs = pool.tile([P, ch + 2 - (ch % 2), 136], F16, tag="rhs")
                        w8 = pool.tile([P, ch, HEADS], F16, tag="w8")
                        cg = pool.tile([P, ch, HEADS], F16, tag="cg")
                        # self-loop chunk (dense own rows)
                        sA = pool.tile([P, HEADS], F16, tag="sA")
                        nc.scalar.activation(sA[:], own[:, 128:136],
                                             mybir.ActivationFunctionType.Exp,
                                             bias=mshift[:])
                        sC = pool.tile([P, HEADS], F16, tag="sC")
                        nc.scalar.activation(sC[:], own[:, 128:136],
                                             mybir.ActivationFunctionType.Exp,
                                             bias=mshift[:], scale=NEG)
                        nc.vector.tensor_tensor(out=sC[:], in0=sC[:], in1=G8[:],
                                                op=mybir.AluOpType.mult)
                        nc.vector.tensor_tensor(out=rhs[:, ch, 128:136], in0=sA[:],
                                                in1=sC[:], op=mybir.AluOpType.max)
                        nc.vector.tensor_tensor(
                            out=rhs[:, ch, 0:128].rearrange("p (h k) -> p h k", k=HID),
                            in0=own[:, 0:128].rearrange("p (h k) -> p h k", k=HID),
                            in1=rhs[:, ch, 128:136, None].to_broadcast([P, HEADS, HID]),
                            op=mybir.AluOpType.mult)
                        for (xo, ro, ln) in ranges:
                            if ln == 0:
                                continue
                            asx = X[:, xo:xo + ln, 128:136]
                            # w8 <- exp(as - M)
                            nc.scalar.activation(
                                w8[:, ro:ro + ln, :], asx,
                                mybir.ActivationFunctionType.Exp, bias=mshift[:])
                            # cg <- exp(0.2 as - M)
                            nc.scalar.activation(
                                cg[:, ro:ro + ln, :], asx,
                                mybir.ActivationFunctionType.Exp, bias=mshift[:],
                                scale=NEG)
                            # cg *= G8 (per-dst softmax rescale)
                            nc.vector.tensor_tensor(
                                out=cg[:, ro:ro + ln, :], in0=cg[:, ro:ro + ln, :],
                                in1=G8[:, None, :].to_broadcast([P, ln, HEADS]),
                                op=mybir.AluOpType.mult)
                            # w = max(A, Cg) -> rhs[:, :, 128:136]
                            nc.vector.tensor_tensor(
                                out=rhs[:, ro:ro + ln, 128:136],
                                in0=w8[:, ro:ro + ln, :], in1=cg[:, ro:ro + ln, :],
                                op=mybir.AluOpType.max)
                            # wh = h * w (per 16-wide head group)
                            nc.vector.tensor_tensor(
                                out=rhs[:, ro:ro + ln, 0:128].rearrange(
                                    "p c (h k) -> p c h k", k=HID),
                                in0=X[:, xo:xo + ln, 0:128].rearrange(
                                    "p c (h k) -> p c h k", k=HID),
                                in1=rhs[:, ro:ro + ln, 128:136, None].to_broadcast(
                                    [P, ln, HEADS, HID]),
                                op=mybir.AluOpType.mult)

                        # 2 chunks per matmul: acc2 holds two side-by-side
                        # accumulators, summed afterwards on DVE
                        nch = ch + 1
                        if nch % 2:
                            nc.gpsimd.memset(rhs[:, nch, :], 0.0)
                            nch += 1
                        acc2 = pp.tile([P, 272], F32, tag="acc", space="PSUM")
                        for i in range(nch // 2):
                            nc.tensor.matmul(
                                out=acc2[:], lhsT=identf[:],
                                rhs=rhs[:, 2 * i:2 * i + 2, :].rearrange(
                                    "p c f -> p (c f)"),
                                start=(i == 0), stop=(i == nch // 2 - 1))
                        acc = pool.tile([P, 136], F32, tag="accs")
                        nc.vector.tensor_copy(out=acc[:], in_=acc2[:, 0:136])
                        nc.vector.tensor_tensor(out=acc[:], in0=acc[:],
                                                in1=acc2[:, 136:272],
                                                op=mybir.AluOpType.add)

                        recip = pool.tile([P, HEADS], F32, tag="recip")
                        nc.vector.reciprocal(recip[:], acc[:, 128:136])
                        o1 = pool.tile([P, P], F32, tag="o1")
                        nc.vector.tensor_tensor(
                            out=o1[:].rearrange("p (h k) -> p h k", k=HID),
                            in0=acc[:, 0:128].rearrange("p (h k) -> p h k", k=HID),
                            in1=recip[:, :, None].to_broadcast([P, HEADS, HID]),
                            op=mybir.AluOpType.mult)
                        nc.vector.tensor_tensor(out=o1[:], in0=o1[:], in1=b1r[:],
                                                op=mybir.AluOpType.add)
                        vmin = pool.tile([P, P], F32, tag="vmin")
                        nc.vector.tensor_scalar(out=vmin[:], in0=o1[:], scalar1=0.0,
                                                scalar2=None, op0=mybir.AluOpType.min)
                        ev_ = pool.tile([P, P], F32, tag="ev")
                        nc.scalar.activation(ev_[:], vmin[:],
                                             mybir.ActivationFunctionType.Exp)
                        elu = pool.tile([P, P], F32, tag="elu")
                        nc.vector.tensor_scalar(out=elu[:], in0=o1[:], scalar1=0.0,
                                                scalar2=None, op0=mybir.AluOpType.max)
                        nc.vector.tensor_tensor(out=elu[:], in0=elu[:], in1=ev_[:],
                                                op=mybir.AluOpType.add)
                        nc.vector.tensor_scalar(out=elu[:], in0=elu[:], scalar1=-1.0,
                                                scalar2=None, op0=mybir.AluOpType.add)
                        eTp = pp.tile([P, P], F32, tag="eTp", space="PSUM")
                        nc.tensor.transpose(out=eTp[:], in_=elu[:], identity=ident32[:])
                        eT = pool.tile([P, P], F16, tag="eT")
                        nc.vector.tensor_copy(out=eT[:], in_=eTp[:])
                        g2p = pp.tile([P, 18], F32, tag="g2p", space="PSUM")
                        nc.tensor.matmul(out=g2p[:], lhsT=eT[:], rhs=w2e[:],
                                         start=True, stop=True)
                        g2t = pool.tile([P, 18], F16, tag="g2t")
                        nc.vector.tensor_copy(out=g2t[:], in_=g2p[:])
                        nc.sync.dma_start(g2_out[b], g2t[:])
                        evo += ce
                        odo += co

            if reps == 1:
                body()
            else:
                with tc.For_i(0, reps, 1):
                    body()
    nc.finalize()
    return nc


def build_neff_c(cfg, reps=1):
    nc = bacc.Bacc(num_swdge_queues=NQ, dynamic_dma_scratch_size=SCRATCH)
    pair_info = cfg["pair_info"]
    si_cols = cfg["si_cols"]
    g_d = nc.dram_tensor("g2", [TROWS // 2, 2, CROW], F16, kind="ExternalInput")
    si_d = nc.dram_tensor("si", [128, si_cols], I16, kind="ExternalInput")
    own_d = nc.dram_tensor("own2", [BLOCKS, P, 18], F16, kind="ExternalInput")
    identf_d = nc.dram_tensor("identf", [P, P], F16, kind="ExternalInput")
    b2r_d = nc.dram_tensor("b2r", [P, CLASSES], F32, kind="ExternalInput")
    out_d = nc.dram_tensor("out2", [BLOCKS, P, CLASSES], F32, kind="ExternalOutput")

    qctr = [0]

    def qrr():
        qctr[0] = (qctr[0] + 1) % NQ
        return qctr[0]

    with tile.TileContext(nc) as tc:
        with tc.tile_pool(name="sbuf", bufs=2) as pool, \
             tc.tile_pool(name="psum", bufs=2, space="PSUM") as pp:
            identf = pool.tile([P, P], F16)
            nc.sync.dma_start(identf[:], identf_d[:])
            b2r = pool.tile([P, CLASSES], F32)
            nc.sync.dma_start(b2r[:], b2r_d[:])
            mshift = pool.tile([P, 1], F32)
            nc.gpsimd.memset(mshift[:], -M_SHIFT)

            def body():
                for info in pair_info:
                    tot_ch = (info["n_ev"] + info["n_od"]) // P
                    ev_ch = info["n_ev"] // P
                    X = pool.tile([P, tot_ch, CROW], F16, tag="X")
                    si = pool.tile([P, (info["n_ev"] + info["n_od"]) // 16], I16, tag="si")
                    nc.sync.dma_start(si[:], si_d[:, info["col0"]:info["col0"] + si.shape[1]])
                    _gather_sections(nc, qrr, X, g_d, si, info)

                    evo = 0
                    odo = ev_ch
                    for i, b in enumerate(info["blocks"]):
                        ce, co = info["ev"][i], info["od"][i]
                        ch = ce + co
                        ranges = [(evo, 0, ce), (odo, ce, co)]
                        own = pool.tile([P, 18], F16, tag="own")
                        nc.sync.dma_start(own[:], own_d[b])
                        G1 = pool.tile([P, 1], F16, tag="G1")
                        nc.scalar.activation(G1[:], own[:, 17:18],
                                             mybir.ActivationFunctionType.Exp,
                                             scale=-0.8)
                        rhs = pool.tile([P, ch + 5 - (ch + 1) % 4 if (ch + 1) % 4
                                         else ch + 1, 17], F16, tag="rhs")
                        w1t = pool.tile([P, ch, 1], F16, tag="w1t")
                        cg = pool.tile([P, ch, 1], F16, tag="cg")
                        sA = pool.tile([P, 1], F16, tag="sA")
                        nc.scalar.activation(sA[:], own[:, 16:17],
                                             mybir.ActivationFunctionType.Exp,
                                             bias=mshift[:])
                        sC = pool.tile([P, 1], F16, tag="sC")
                        nc.scalar.activation(sC[:], own[:, 16:17],
                                             mybir.ActivationFunctionType.Exp,
                                             bias=mshift[:], scale=NEG)
                        nc.vector.tensor_tensor(out=sC[:], in0=sC[:], in1=G1[:],
                                                op=mybir.AluOpType.mult)
                        nc.vector.tensor_tensor(out=rhs[:, ch, 16:17], in0=sA[:],
                                                in1=sC[:], op=mybir.AluOpType.max)
                        nc.vector.tensor_tensor(
                            out=rhs[:, ch, 0:16], in0=own[:, 0:16],
                            in1=rhs[:, ch, 16:17].to_broadcast([P, 16]),
                            op=mybir.AluOpType.mult)
                        for (xo, ro, ln) in ranges:
                            if ln == 0:
                                continue
                            asx = X[:, xo:xo + ln, 16:17]
                            nc.scalar.activation(
                                w1t[:, ro:ro + ln, :], asx,
                                mybir.ActivationFunctionType.Exp, bias=mshift[:])
                            nc.scalar.activation(
                                cg[:, ro:ro + ln, :], asx,
                                mybir.ActivationFunctionType.Exp, bias=mshift[:],
                                scale=NEG)
                            nc.vector.tensor_tensor(
                                out=cg[:, ro:ro + ln, :], in0=cg[:, ro:ro + ln, :],
                                in1=G1[:, None, :].to_broadcast([P, ln, 1]),
                                op=mybir.AluOpType.mult)
                            nc.vector.tensor_tensor(
                                out=rhs[:, ro:ro + ln, 16:17],
                                in0=w1t[:, ro:ro + ln, :], in1=cg[:, ro:ro + ln, :],
                                op=mybir.AluOpType.max)
                            nc.vector.tensor_tensor(
                                out=rhs[:, ro:ro + ln, 0:16],
                                in0=X[:, xo:xo + ln, 0:16],
                                in1=rhs[:, ro:ro + ln, 16:17].to_broadcast([P, ln, 16]),
                                op=mybir.AluOpType.mult)

                        # 4 chunks per matmul, tree-summed afterwards
                        nch = ch + 1
                        if nch % 4:
                            nc.gpsimd.memset(rhs[:, nch:nch + (4 - nch % 4), :], 0.0)
                            nch += 4 - nch % 4
                        acc4 = pp.tile([P, 68], F32, tag="acc", space="PSUM")
                        for i in range(nch // 4):
                            nc.tensor.matmul(
                                out=acc4[:], lhsT=identf[:],
                                rhs=rhs[:, 4 * i:4 * i + 4, :].rearrange(
                                    "p c f -> p (c f)"),
                                start=(i == 0), stop=(i == nch // 4 - 1))
                        acc34 = pool.tile([P, 34], F32, tag="acc34")
                        nc.vector.tensor_copy(out=acc34[:], in_=acc4[:, 0:34])
                        nc.vector.tensor_tensor(out=acc34[:], in0=acc34[:],
                                                in1=acc4[:, 34:68],
                                                op=mybir.AluOpType.add)
                        acc = pool.tile([P, 17], F32, tag="accs")
                        nc.vector.tensor_tensor(out=acc[:], in0=acc34[:, 0:17],
                                                in1=acc34[:, 17:34],
                                                op=mybir.AluOpType.add)

                        recip = pool.tile([P, 1], F32, tag="recip")
                        nc.vector.reciprocal(recip[:], acc[:, 16:17])
                        o2 = pool.tile([P, CLASSES], F32, tag="o2")
                        nc.vector.tensor_tensor(
                            out=o2[:], in0=acc[:, 0:16],
                            in1=recip[:].to_broadcast([P, CLASSES]),
                            op=mybir.AluOpType.mult)
                        nc.vector.tensor_tensor(out=o2[:], in0=o2[:], in1=b2r[:],
                                                op=mybir.AluOpType.add)
                        nc.sync.dma_start(out_d[b], o2[:])
                        evo += ce
                        odo += co

            if reps == 1:
                body()
            else:
                with tc.For_i(0, reps, 1):
                    body()
    nc.finalize()
    return nc


# ------------------------------ runner plumbing ------------------------------

def make_runner(nc, n_cores=N_CORES):
    import jax
    from jax.sharding import Mesh, PartitionSpec
    from jax.experimental.shard_map import shard_map
    from concourse.bass2jax import _bass_exec_p, install_neuronx_cc_hook, partition_id_tensor

    install_neuronx_cc_hook()
    partition_name = nc.partition_id_tensor.name if nc.partition_id_tensor else None
    in_names, out_names, out_avals = [], [], []
    for alloc in nc.m.functions[0].allocations:
        if not isinstance(alloc, mybir.MemoryLocationSet):
            continue
        name = alloc.memorylocations[0].name
        if alloc.kind == "ExternalInput":
            if name != partition_name:
                in_names.append(name)
        elif alloc.kind == "ExternalOutput":
            out_names.append(name)
            out_avals.append(jax.core.ShapedArray(tuple(alloc.tensor_shape),
                                                  mybir.dt.np(alloc.dtype)))
    n_params = len(in_names)
    all_names = in_names + out_names + ([partition_name] if partition_name else [])

    def _body(*args):
        operands = list(args)
        if partition_name is not None:
            operands.append(partition_id_tensor())
        return tuple(_bass_exec_p.bind(
            *operands, out_avals=tuple(out_avals), in_names=tuple(all_names),
            out_names=tuple(out_names), lowering_input_output_aliases=(),
            sim_require_finite=False, sim_require_nnan=False, nc=nc))

    devices = jax.devices()[:n_cores]
    mesh = Mesh(np.asarray(devices), ("core",))
    sharded = jax.jit(
        shard_map(_body, mesh=mesh,
                  in_specs=(PartitionSpec("core"),) * (n_params + len(out_names)),
                  out_specs=(PartitionSpec("core"),) * len(out_names),
                  check_rep=False),
        keep_unused=True)

    import jax as _jax
    from jax.sharding import NamedSharding

    _dev_args = {}

    def run(in_maps, key=None, raw=False):
        if key is not None and key in _dev_args:
            args = _dev_args[key]
        else:
            concat_in = [np.concatenate([np.asarray(m[nm]) for m in in_maps], axis=0)
                         for nm in in_names]
            concat_zero = [np.zeros((n_cores * a.shape[0], *a.shape[1:]), a.dtype)
                           for a in out_avals]
            sh = NamedSharding(mesh, PartitionSpec("core"))
            args = [_jax.device_put(a, sh) for a in concat_in + concat_zero]
            _jax.block_until_ready(args)
            if key is not None:
                _dev_args[key] = args
        outs = sharded(*args)
        _jax.block_until_ready(outs)
        if raw:
            return outs
        return [
            {nm: np.asarray(outs[i]).reshape(n_cores, *out_avals[i].shape)[c]
             for i, nm in enumerate(out_names)}
            for c in range(n_cores)
        ]

    return run


def _get_compiled(key, builder):
    if key not in _cache:
        nc = builder()
        _cache[key] = make_runner(nc)
    return _cache[key]


# --------------------------------- kernel ------------------------------------

def kernel(x, edge_index, W1, a_src1, a_dst1, b1, W2, a_src2, a_dst2, b2):
    x = np.asarray(x, np.float32)
    edge_index = np.asarray(edge_index)
    W1 = np.asarray(W1, np.float32)
    W2 = np.asarray(W2, np.float32)
    a_src1 = np.asarray(a_src1, np.float32)
    a_dst1 = np.asarray(a_dst1, np.float32)
    a_src2 = np.asarray(a_src2, np.float32)
    a_dst2 = np.asarray(a_dst2, np.float32)
    b1 = np.asarray(b1, np.float32)
    b2 = np.asarray(b2, np.float32)

    cfg = host_prep(edge_index)
    nos = cfg["node_of_slot"]

    As = np.zeros((P, HEADS), np.float32)
    Ad = np.zeros((P, HEADS), np.float32)
    for h in range(HEADS):
        As[h * HID:(h + 1) * HID, h] = a_src1[h]
        Ad[h * HID:(h + 1) * HID, h] = a_dst1[h]
    W1ext = np.concatenate([W1, W1 @ As, W1 @ Ad], 1).astype(np.float16)
    W2ext = np.concatenate([W2, W2 @ a_src2.T, W2 @ a_dst2.T], 1).astype(np.float16)
    identf = np.eye(P, dtype=np.float16)
    ident32 = np.eye(P, dtype=np.float32)
    b1r = np.ascontiguousarray(np.broadcast_to(b1, (P, P))).astype(np.float32)
    b2r = np.ascontiguousarray(np.broadcast_to(b2, (P, CLASSES))).astype(np.float32)

    xfull = np.zeros((N_PAD, F_IN), np.float32)
    xfull[:N] = x
    xp = xfull[nos]                                    # [slot, feat]
    xT = np.ascontiguousarray(xp.T).astype(np.float16)  # [feat, slot]

    # ---- NEFF-A ----
    run_a = _get_compiled("A", build_neff_a)
    in_a = [{"xT": np.ascontiguousarray(xT[:, k * NODES_PER_CORE:(k + 1) * NODES_PER_CORE]),
             "w1e": W1ext} for k in range(N_CORES)]
    res_a = run_a(in_a)
    g_host = np.concatenate([res_a[k]["g_out"] for k in range(N_CORES)], axis=0)

    # ---- build B table ----
    gB = np.zeros((TROWS, BROW), np.float16)
    gB[:N_PAD, 0:144] = g_host
    gB[PAD_EVEN, 128:136] = -100.0
    gB[PAD_ODD, 128:136] = -100.0
    gB[cfg["copy_rows"]] = gB[cfg["copy_slots"]]
    gBp = gB.reshape(TROWS // 2, 2, BROW)

    lane_slot = cfg["lane_slot"]                       # [8, 49, 128]

    # ---- NEFF-B ----
    run_b = _get_compiled(("B", cfg["si_cols"]), lambda: build_neff_b(cfg))
    in_b = [{"g": gBp, "si": cfg["si_all"][k],
             "own": np.ascontiguousarray(g_host[lane_slot[k].reshape(-1)]
                                         ).reshape(BLOCKS, P, 144),
             "identf": identf, "ident32": ident32, "b1r": b1r, "w2e": W2ext}
            for k in range(N_CORES)]
    res_b = run_b(in_b)
    g2_host = np.zeros((N_PAD, 18), np.float16)
    for k in range(N_CORES):
        g2_host[lane_slot[k].reshape(-1)] = res_b[k]["g2_out"].reshape(-1, 18)
    bad = ~np.isfinite(g2_host.astype(np.float32)).all(1)
    g2_host[bad] = 0

    # ---- build C table ----
    gC = np.zeros((TROWS, CROW), np.float16)
    gC[:N_PAD, 0:18] = g2_host
    gC[PAD_EVEN, 16] = -100.0
    gC[PAD_ODD, 16] = -100.0
    gC[cfg["copy_rows"]] = gC[cfg["copy_slots"]]
    gCp = gC.reshape(TROWS // 2, 2, CROW)

    # ---- NEFF-C ----
    run_c = _get_compiled(("C", cfg["si_cols"]), lambda: build_neff_c(cfg))
    in_c = [{"g2": gCp, "si": cfg["si_all"][k],
             "own2": np.ascontiguousarray(g2_host[lane_slot[k].reshape(-1)]
                                          ).reshape(BLOCKS, P, 18),
             "identf": identf, "b2r": b2r} for k in range(N_CORES)]
    res_c = run_c(in_c)

    out_slots = np.zeros((N_PAD, CLASSES), np.float32)
    for k in range(N_CORES):
        out_slots[lane_slot[k].reshape(-1)] = res_c[k]["out2"].reshape(-1, CLASSES)
    out = out_slots[cfg["slot_of_node"]]
    global _last_cfg, _last_inputs
    _last_cfg = cfg
    _last_inputs = {"A": in_a, "B": in_b, "C": in_c}
    return out[:N].astype(np.float32)
